# revision 1
# baseline (speedup 1.0000x reference)
"""Causal self-attention (B=2, T=2048, C=1024, H=16) on 8 TRN2 NeuronCores.

Sharding: core = b * 4 + g  (b in 0..1 batches, g in 0..3 head-groups of 4 heads).
Each core computes qkv projection for its 4 heads, causal flash-style attention,
and the output projection restricted to its heads' rows of w_proj, producing a
partial y[b] (bf16). Host sums the 4 partials per batch and folds in the exact
bias terms (b_qk applied on device; b_v and b_proj folded algebraically on host:
att rows sum to 1 so  att@(v + 1 b_v^T) @ w_p = att@v@w_p + b_v@w_p).

Device compute is bf16 matmuls with fp32 PSUM accumulation:
 - q^T/k^T in head-pair-stacked layout [128, T]; v in natural [T, 256] layout
 - S^T blocks via K=64 row-packed matmul pairs (tile_position rows 0/64)
 - exp on ACT with exact causal column regions; triangular-mask mul on DVE
 - AV (O^T) and row-sums l via col-packed matmuls accumulated over k-blocks
 - softmax normalization deferred to O: 1/l broadcast via a selection matmul
 - projection back to natural [T, C] layout (lhsT = O^T blocks), DMA out bf16

Pipeline: the kernel runs as one software pipeline over 512-token chunks R —
chunk R's attention (ACT-exp paced) is interleaved with chunk R+1's qkv
projection and chunk R-1's output projection as "background" PE work, so the
TensorEngine never idles. PSUM (8 banks): S pair-tiles [128,1024] x2 bufs (4),
O^T pair accumulators x2 (2), l accumulator (1, reused for the 1/l broadcast),
and one rotating bank for qkv/proj background pieces.
"""

import sys

if "/opt/trn_rl_repo" not in sys.path:
    sys.path.insert(0, "/opt/trn_rl_repo")

import numpy as np
import ml_dtypes

BF16 = ml_dtypes.bfloat16
B, T_FULL, C = 2, 2048, 1024
H, HD = 16, 64
HPC = 4  # heads per core
NCORES = 8
NK = C // 128  # contraction k-tiles


def build_nc(T, num_devices=NCORES, interleave=True):
    import concourse.bass as bass
    import concourse.tile as tile
    from concourse import bacc, mybir

    bf = mybir.dt.bfloat16
    f32 = mybir.dt.float32
    NT = T // 128   # token tiles
    NCH = T // 512  # token chunks

    nc = bacc.Bacc("TRN2", target_bir_lowering=False, debug=False,
                   num_devices=num_devices)

    xT_d = nc.dram_tensor("xT", [C, T], bf, kind="ExternalInput")
    wqkv_d = nc.dram_tensor("wqkv", [C, 768], bf, kind="ExternalInput")
    wp_d = nc.dram_tensor("wp", [256, C], bf, kind="ExternalInput")
    bqk_d = nc.dram_tensor("bqk", [128, 4], f32, kind="ExternalInput")
    cst_d = nc.dram_tensor("cst", [128, 416], bf, kind="ExternalInput")
    y_d = nc.dram_tensor("y", [T, C], bf, kind="ExternalOutput")

    Exp = mybir.ActivationFunctionType.Exp
    PSUM = bass.MemorySpace.PSUM

    with tile.TileContext(nc) as tc, nc.allow_low_precision(
            reason="bf16 activations by design; fp32 PSUM accumulation"):
        with (
            tc.tile_pool(name="const", bufs=1) as cpool,
            tc.tile_pool(name="act", bufs=1) as apool,
            tc.tile_pool(name="se", bufs=5) as sepool,
            tc.tile_pool(name="small", bufs=2) as spool,
            tc.tile_pool(name="ysb", bufs=6) as ypool,
        ):
            # ---- constant/weight loads ----
            # order: qkv weights + chunk-0 x first so PE starts early
            xT = [cpool.tile([128, T], bf, tag=f"xT{k}", name=f"xT{k}")
                  for k in range(NK)]
            wqkv = [cpool.tile([128, 768], bf, tag=f"wqkv{k}", name=f"wqkv{k}")
                    for k in range(NK)]
            wqk = [t[:, 0:512] for t in wqkv]
            wv = [t[:, 512:768] for t in wqkv]
            wp = [cpool.tile([128, C], bf, tag=f"wp{k}", name=f"wp{k}")
                  for k in range(2)]
            xh = min(1024, T)
            for k in range(NK):
                nc.sync.dma_start(wqkv[k][:], wqkv_d.ap()[128 * k:128 * (k + 1), :])
                nc.sync.dma_start(
                    xT[k][:, 0:xh], xT_d.ap()[128 * k:128 * (k + 1), 0:xh])
            bqk = cpool.tile([128, 4], f32, tag="bqk", name="bqk")
            nc.sync.dma_start(bqk[:], bqk_d.ap()[:])
            cst = cpool.tile([128, 416], bf, tag="cst", name="cst")
            nc.sync.dma_start(cst[:], cst_d.ap()[:])
            tri = cst[:, 0:128]
            ones1 = cst[:, 128:160]
            sel = cst[:, 160:416]
            if xh < T:
                for k in range(NK):
                    nc.sync.dma_start(
                        xT[k][:, xh:T], xT_d.ap()[128 * k:128 * (k + 1), xh:T])
            for k in range(2):
                nc.sync.dma_start(wp[k][:], wp_d.ap()[128 * k:128 * (k + 1), :])

            # ---- persistent activations ----
            # qk_sb[m]: m=0 q(pair ab), 1 q(pair cd), 2 k(ab), 3 k(cd)
            qk_sb = [apool.tile([128, T], bf, tag=f"qk{m}", name=f"qk{m}") for m in range(4)]
            v_sb = [apool.tile([128, 256], bf, tag=f"v{t}", name=f"v{t}") for t in range(NT)]
            O_sb = [apool.tile([128, T], bf, tag=f"O{p}", name=f"O{p}") for p in range(2)]

            # ---- single PSUM layout for all phases (8 banks):
            #   "S" x2 bufs [128,1024] = 4 banks (S blocks + qkv chunk psums)
            #   Oab, Ocd, l = 3 banks;  y = 1 bank (projection)
            with (
                tc.tile_pool(name="ps_s", bufs=2, space=PSUM) as ps_s,
                tc.tile_pool(name="ps_o", bufs=1, space=PSUM) as ps_o,
                tc.tile_pool(name="ps_y", bufs=1, space=PSUM) as ps_y,
            ):
                def emit_qkv_piece(c, piece, filler=False):
                    """piece 0..3: q/k m-tile; 4..7: v token-tile."""
                    pool = ps_y if filler else ps_s
                    tag = "y" if filler else "S"
                    if piece < 4:
                        m = piece
                        pt = pool.tile([128, 512], f32, tag=tag, name="qkp")
                        for k in range(NK):
                            nc.tensor.matmul(
                                pt[:],
                                wqk[k][:, 128 * m:128 * (m + 1)],
                                xT[k][:, 512 * c:512 * (c + 1)],
                                start=(k == 0), stop=(k == NK - 1),
                            )
                        nc.vector.tensor_scalar_add(
                            qk_sb[m][:, 512 * c:512 * (c + 1)], pt[:],
                            bqk[:, m:m + 1],
                        )
                    else:
                        tt = 4 * c + piece - 4
                        pv = pool.tile([128, 256], f32, tag=tag, name="vp")
                        for k in range(NK):
                            nc.tensor.matmul(
                                pv[:],
                                xT[k][:, 128 * tt:128 * (tt + 1)],
                                wv[k],
                                start=(k == 0), stop=(k == NK - 1),
                            )
                        nc.vector.tensor_copy(v_sb[tt][:], pv[:])

                def emit_proj_piece(R, piece, pool=None, tag="y"):
                    tt = 4 * R + piece // 2
                    cc = piece % 2
                    yp = (pool or ps_y).tile([128, 512], f32, tag=tag, name="y")
                    for kd in range(2):
                        nc.tensor.matmul(
                            yp[:],
                            O_sb[kd][:, 128 * tt:128 * (tt + 1)],
                            wp[kd][:, 512 * cc:512 * (cc + 1)],
                            start=(kd == 0), stop=(kd == 1),
                        )
                    ysb = ypool.tile([128, 512], bf, tag="ysb", name="ysb")
                    nc.vector.tensor_copy(ysb[:], yp[:])
                    nc.sync.dma_start(
                        y_d.ap()[128 * tt:128 * (tt + 1),
                                 512 * cc:512 * (cc + 1)],
                        ysb[:])

                # qkv for chunk 0 up front; later chunks + projections are
                # interleaved into the attention loop as background pieces to
                # keep PE continuously busy during ACT-paced sections.
                nhead = min(1, NCH)
                for c in range(nhead):
                    for piece in range(8):
                        emit_qkv_piece(c, piece)
                if not interleave:
                    for c in range(nhead, NCH):
                        for piece in range(8):
                            emit_qkv_piece(c, piece)

                bg = []  # deferred (fn, args) pieces
                for R in range(NCH):
                    if interleave and R + nhead < NCH:
                        bg.extend(("qkv", R + nhead, p) for p in range(8))
                    if interleave and NCH == 4:
                        # deadline-based: defer early projections so the
                        # filler-starved late chunks get background PE work
                        if R == 2:
                            bg.extend(("proj", 0, p) for p in range(8))
                        elif R == 3:
                            bg.extend(("proj", rr, p) for rr in (1, 2)
                                      for p in range(8))
                    O_ps = [ps_o.tile([128, 512], f32, tag="Oab", name="Oab"),
                            ps_o.tile([128, 512], f32, tag="Ocd", name="Ocd")]
                    lps = ps_o.tile([128, 512], f32, tag="l", name="l")
                    njr = 4 * R + 4
                    nbg0 = len(bg)
                    emitted = 0
                    for j in range(njr):
                        m = j - 4 * R
                        lo = 128 * m if m >= 0 else 0
                        last = (j == njr - 1)
                        st = (j == 0)
                        Ses = []
                        gates = []
                        for pi in range(2):
                            qT = qk_sb[pi]
                            kT = qk_sb[2 + pi]
                            Sp = ps_s.tile([128, 1024], f32, tag="S", name="S")
                            # S^T block: row-packed K=64 pair (heads 2pi, 2pi+1)
                            nc.tensor.matmul(
                                Sp[:, lo:512],
                                kT[0:64, 128 * j:128 * (j + 1)],
                                qT[0:64, 512 * R + lo:512 * (R + 1)],
                                start=True, stop=True,
                            )
                            nc.tensor.matmul(
                                Sp[:, 512 + lo:1024],
                                kT[64:128, 128 * j:128 * (j + 1)],
                                qT[64:128, 512 * R + lo:512 * (R + 1)],
                                start=True, stop=True,
                            )
                            Se = sepool.tile([128, 1024], bf, tag="Se", name="Se")
                            sp3 = Sp.rearrange("p (h n) -> p h n", h=2)
                            se3 = Se.rearrange("p (h n) -> p h n", h=2)
                            expi = nc.scalar.activation(
                                se3[:, :, lo:512], sp3[:, :, lo:512], Exp,
                                scale=0.125,
                            )
                            gate = None
                            if m >= 0:
                                # diagonal 128-block: upper-tri (incl diag) mask
                                nc.vector.tensor_mul(
                                    Se[:, lo:lo + 128], Se[:, lo:lo + 128], tri)
                                gate = nc.vector.tensor_mul(
                                    Se[:, 512 + lo:512 + lo + 128],
                                    Se[:, 512 + lo:512 + lo + 128], tri)
                            # AV: col-packed M=64 pair into O^T psum;
                            # gate the first on the pair's second mask so both
                            # become ready together (concurrent col groups)
                            av1 = nc.tensor.matmul(
                                O_ps[pi][0:64, lo:512],
                                v_sb[j][:, 128 * pi:128 * pi + 64],
                                Se[:, lo:512],
                                start=st, stop=last, tile_position=(0, 0),
                                skip_group_check=True,
                            )
                            if gate is not None:
                                from concourse.tile import add_dep_helper as _adh
                                _adh(av1.ins, gate.ins,
                                     reason="group AV col pair")
                            nc.tensor.matmul(
                                O_ps[pi][64:128, lo:512],
                                v_sb[j][:, 128 * pi + 64:128 * (pi + 1)],
                                Se[:, 512 + lo:1024],
                                start=st, stop=last, tile_position=(0, 64),
                                skip_group_check=True,
                            )
                            Ses.append(Se)
                            gates.append(gate if gate is not None else expi)
                        # l rows (replicated x32), all 4 heads emitted
                        # adjacently -> 4 concurrent col-group matmuls on HW
                        from concourse.tile import add_dep_helper

                        def _unwrap(i):
                            return getattr(i, "ins", i)
                        for pi in range(2):
                            l1 = nc.tensor.matmul(
                                lps[64 * pi:64 * pi + 32, lo:512],
                                ones1, Ses[pi][:, lo:512],
                                start=st, stop=last, tile_position=(0, 64 * pi),
                                skip_group_check=True,
                            )
                            l2 = nc.tensor.matmul(
                                lps[64 * pi + 32:64 * pi + 64, lo:512],
                                ones1, Ses[pi][:, 512 + lo:1024],
                                start=st, stop=last,
                                tile_position=(0, 64 * pi + 32),
                                skip_group_check=True,
                            )
                            if pi == 0:
                                add_dep_helper(_unwrap(l1), _unwrap(gates[1]),
                                               reason="group 4-way l matmuls")
                                add_dep_helper(_unwrap(l2), _unwrap(gates[1]),
                                               reason="group 4-way l matmuls")
                        # spread background pieces (next chunk's qkv, earlier
                        # chunks' projections) across the attention loop --
                        # emitted after the j-group so pair matmuls outrank
                        # fillers in scheduler priority
                        want = (nbg0 * (j + 1) + njr - 1) // njr
                        while emitted < want and bg:
                            kind, rr, p = bg.pop(0)
                            emitted += 1
                            if kind == "qkv":
                                emit_qkv_piece(rr, p, filler=True)
                            else:
                                emit_proj_piece(rr, p)
                    # normalization: rl = 1/l on anchor rows, broadcast by matmul
                    rl = spool.tile([128, 512], bf, tag="rl", name="rl")
                    nc.vector.reciprocal(rl[:], lps[:])
                    for pi in range(2):
                        bcp = ps_o.tile([128, 512], f32, tag="l", name="l")
                        nc.tensor.matmul(
                            bcp[:], sel[:, 128 * pi:128 * (pi + 1)], rl[:],
                            start=True, stop=True,
                        )
                        bcs = spool.tile([128, 512], bf, tag="bcs", name="bcs")
                        nc.vector.tensor_copy(bcs[:], bcp[:])
                        nc.vector.tensor_mul(
                            O_sb[pi][:, 512 * R:512 * (R + 1)], O_ps[pi][:],
                            bcs[:])
                    # projection for this chunk rides a later chunk's loop
                    if interleave:
                        if NCH != 4 or R == NCH - 1:
                            bg.extend(("proj", R, p) for p in range(8))
                    else:
                        for p in range(8):
                            emit_proj_piece(R, p)

                # flush remaining pieces across the now-free PSUM tags so the
                # tail pipelines instead of serializing on one bank
                flush_slots = [(ps_y, "y"), (ps_s, "S"), (ps_o, "Oab"),
                               (ps_o, "Ocd"), (ps_o, "l")]
                for i, (kind, rr, p) in enumerate(bg):
                    pool, tag = flush_slots[i % len(flush_slots)]
                    if kind == "qkv":
                        emit_qkv_piece(rr, p, filler=True)
                    else:
                        emit_proj_piece(rr, p, pool=pool, tag=tag)

    nc.compile()
    return nc


def make_core_inputs(x, w_qkv, b_qkv, w_proj, core, T=None):
    """Host-side shard/prep for one core. Returns the in_map dict."""
    if T is None:
        T = x.shape[1]
    b, g = divmod(core, 4)
    heads = [4 * g + i for i in range(HPC)]

    xT = np.ascontiguousarray(np.asarray(x[b], np.float32).T).astype(BF16)

    qcols = [w_qkv[:, h * HD:(h + 1) * HD] for h in heads]
    kcols = [w_qkv[:, C + h * HD:C + (h + 1) * HD] for h in heads]
    vcols = [w_qkv[:, 2 * C + h * HD:2 * C + (h + 1) * HD] for h in heads]
    wqk = np.concatenate(qcols + kcols, axis=1).astype(BF16)      # [C, 512]
    wv = np.concatenate(vcols, axis=1).astype(BF16)               # [C, 256]
    wp = np.concatenate([w_proj[h * HD:(h + 1) * HD, :] for h in heads],
                        axis=0).astype(BF16)                      # [256, C]

    bq = [b_qkv[h * HD:(h + 1) * HD] for h in heads]
    bk = [b_qkv[C + h * HD:C + (h + 1) * HD] for h in heads]
    bqk = np.concatenate(bq + bk).astype(np.float32).reshape(4, 128).T
    bqk = np.ascontiguousarray(bqk)                               # [128, 4]

    a = np.arange(128)
    tri = (a[:, None] <= a[None, :]).astype(BF16)                 # [128, 128]
    ones1 = np.ones((128, 32), dtype=BF16)
    sel = np.zeros((128, 256), dtype=BF16)
    sel[0, 0:64] = 1      # pair ab: out rows 0:64   <- l row 0 (head 0)
    sel[32, 64:128] = 1   #          out rows 64:128 <- l row 32 (head 1)
    sel[64, 128:192] = 1  # pair cd: out rows 0:64   <- l row 64 (head 2)
    sel[96, 192:256] = 1  #          out rows 64:128 <- l row 96 (head 3)

    return {
        "xT": xT, "wqkv": np.concatenate([wqk, wv], axis=1),
        "wp": wp, "bqk": bqk,
        "cst": np.concatenate([tri, ones1, sel], axis=1),
    }


_compiled = {}


def _get_nc(T):
    if T not in _compiled:
        _compiled[T] = build_nc(T)
    return _compiled[T]


def kernel(x, w_qkv, b_qkv, w_proj, b_proj):
    from concourse.bass_utils import run_bass_kernel_spmd

    x = np.asarray(x, np.float32)
    w_qkv = np.asarray(w_qkv, np.float32)
    b_qkv = np.asarray(b_qkv, np.float32)
    w_proj = np.asarray(w_proj, np.float32)
    b_proj = np.asarray(b_proj, np.float32)
    T = x.shape[1]

    nc = _get_nc(T)
    in_maps = [make_core_inputs(x, w_qkv, b_qkv, w_proj, core, T)
               for core in range(NCORES)]
    res = run_bass_kernel_spmd(nc, in_maps, core_ids=list(range(NCORES)))

    y = np.zeros((B, T, C), np.float32)
    for core in range(NCORES):
        b = core // 4
        y[b] += res.results[core]["y"].astype(np.float32)
    y += b_proj[None, None, :] + (b_qkv[2 * C:3 * C] @ w_proj)[None, None, :]
    return y



# revision 4
# speedup vs baseline: 1.2878x; 1.2878x over previous
"""Causal self-attention (B=2, T=2048, C=1024, H=16) on 8 TRN2 NeuronCores.

Sharding: core = b * 4 + g  (b in 0..1 batches, g in 0..3 head-groups of 4 heads).
Each core computes qkv projection for its 4 heads, causal attention, and the
output projection restricted to its heads' rows of w_proj, producing a partial
y[b] (bf16). Host sums the 4 partials per batch and folds in the exact bias
terms (b_qk applied on device; b_v and b_proj folded algebraically on host:
att rows sum to 1 so  att@(v + 1 b_v^T) @ w_p = att@v@w_p + b_v@w_p).

Structure (cost-model-aware: PE matmul time = out-free-size only):
 - q^T/k^T in head-pair-stacked layout [128, T]; v' in [T, 4x(64+1)] layout
   where the extra per-head column of the AV rhs is a constant, so the AV
   matmul also produces the softmax row-sums l for free.
 - attention runs per 256-query superblock I over key blocks j<=2I+1:
   S^T block [keys 128, 4 heads x 256 q] via K=64 matmuls; exp on ACT
   (one 1024-wide instr); triangular-mask mul on DVE for diagonal blocks;
   AV "flipped": out O[q, hd] with lhsT=Se block, rhs=v' (free 65), PSUM
   accumulated over j into per-q-block O tiles (pre-zeroed by memset so
   independent per-head groups can share a PSUM bank with start=False).
 - softmax normalization: per-partition scalar mul (1/l) on DVE -> O_sb,
   then PE-transpose of O to O^T layout for the projection (lhsT), which
   streams y [tok, C] pieces to DRAM in bf16.
 - qkv projection pieces and output projection pieces are interleaved into
   the attention loop as background PE work so the TensorEngine never idles
   while ACT works through the exps.
"""

import sys

if "/opt/trn_rl_repo" not in sys.path:
    sys.path.insert(0, "/opt/trn_rl_repo")

import numpy as np
import ml_dtypes

BF16 = ml_dtypes.bfloat16
B, T_FULL, C = 2, 2048, 1024
H, HD = 16, 64
HPC = 4  # heads per core
NCORES = 8
NK = C // 128  # contraction k-tiles


def build_nc(T, num_devices=NCORES):
    import concourse.bass as bass
    import concourse.tile as tile
    from concourse import bacc, mybir

    bf = mybir.dt.bfloat16
    f32 = mybir.dt.float32
    NT = T // 128   # token tiles
    NSB = T // 256  # query superblocks
    NCH = T // 512  # token chunks (qkv piece granularity)

    nc = bacc.Bacc("TRN2", target_bir_lowering=False, debug=False,
                   num_devices=num_devices)

    xT_d = nc.dram_tensor("xT", [C, T], bf, kind="ExternalInput")
    wqkv_d = nc.dram_tensor("wqkv", [C, 768], bf, kind="ExternalInput")
    wp_d = nc.dram_tensor("wp", [256, C], bf, kind="ExternalInput")
    bqk_d = nc.dram_tensor("bqk", [128, 4], f32, kind="ExternalInput")
    cst_d = nc.dram_tensor("cst", [128, 256], bf, kind="ExternalInput")
    y_d = nc.dram_tensor("y", [T, C], bf, kind="ExternalOutput")

    Exp = mybir.ActivationFunctionType.Exp
    PSUM = bass.MemorySpace.PSUM

    with tile.TileContext(nc) as tc, nc.allow_low_precision(
            reason="bf16 activations by design; fp32 PSUM accumulation"):
        with (
            tc.tile_pool(name="const", bufs=1) as cpool,
            tc.tile_pool(name="act", bufs=1) as apool,
            tc.tile_pool(name="se", bufs=3) as sepool,
            tc.tile_pool(name="osb", bufs=3) as ospool,
            tc.tile_pool(name="small", bufs=2) as spool,
            tc.tile_pool(name="ysb", bufs=4) as ypool,
        ):
            # ---- constant/weight loads: chunk-0 x + qkv weights first ----
            xT = [cpool.tile([128, T], bf, tag=f"xT{k}", name=f"xT{k}")
                  for k in range(NK)]
            wqkv = [cpool.tile([128, 768], bf, tag=f"wqkv{k}", name=f"wqkv{k}")
                    for k in range(NK)]
            wqk = [t[:, 0:512] for t in wqkv]
            wv = [t[:, 512:768] for t in wqkv]
            wp = [cpool.tile([128, C], bf, tag=f"wp{k}", name=f"wp{k}")
                  for k in range(2)]
            for k in range(NK):
                nc.sync.dma_start(wqkv[k][:], wqkv_d.ap()[128 * k:128 * (k + 1), :])
                nc.sync.dma_start(
                    xT[k][:, 0:512], xT_d.ap()[128 * k:128 * (k + 1), 0:512])
            bqk = cpool.tile([128, 4], f32, tag="bqk", name="bqk")
            nc.sync.dma_start(bqk[:], bqk_d.ap()[:])
            cst = cpool.tile([128, 256], bf, tag="cst", name="cst")
            nc.sync.dma_start(cst[:], cst_d.ap()[:])
            tri = cst[:, 0:128]
            ident = cst[:, 128:256]
            for lo in range(512, T, 512):
                for k in range(NK):
                    nc.sync.dma_start(
                        xT[k][:, lo:lo + 512],
                        xT_d.ap()[128 * k:128 * (k + 1), lo:lo + 512])
            for k in range(2):
                nc.sync.dma_start(wp[k][:], wp_d.ap()[128 * k:128 * (k + 1), :])

            # ---- persistent activations ----
            # qk_sb[m]: m=0 q(pair ab), 1 q(pair cd), 2 k(ab), 3 k(cd)
            qk_sb = [apool.tile([128, T], bf, tag=f"qk{m}", name=f"qk{m}")
                     for m in range(4)]
            # v' tile: per token-tile, 4 heads x (64 v-cols + 1 const col)
            vv = apool.tile([128, NT * 260], bf, tag="vv", name="vv")
            vv4 = vv.rearrange("p (t h c) -> p t h c", t=NT, h=HPC)
            nc.vector.memset(vv4[:, :, :, 64:65], 1.0)
            OT_sb = [apool.tile([128, T], bf, tag=f"OT{k}", name=f"OT{k}")
                     for k in range(2)]

            with (
                tc.tile_pool(name="ps_s", bufs=2, space=PSUM) as ps_s,
                tc.tile_pool(name="ps_o", bufs=1, space=PSUM) as ps_o,
                tc.tile_pool(name="ps_bg", bufs=2, space=PSUM) as ps_bg,
            ):
                def emit_qkv_piece(c, piece):
                    """piece 0..3: q/k m-tile; 4..7: v token-tile."""
                    if piece < 4:
                        m = piece
                        pt = ps_bg.tile([128, 512], f32, tag="bg", name="qkp")
                        for k in range(NK):
                            nc.tensor.matmul(
                                pt[:],
                                wqk[k][:, 128 * m:128 * (m + 1)],
                                xT[k][:, 512 * c:512 * (c + 1)],
                                start=(k == 0), stop=(k == NK - 1),
                            )
                        nc.vector.tensor_scalar_add(
                            qk_sb[m][:, 512 * c:512 * (c + 1)], pt[:],
                            bqk[:, m:m + 1],
                        )
                    else:
                        tt = 4 * c + piece - 4
                        pv = ps_bg.tile([128, 512], f32, tag="bg", name="vp")
                        for k in range(NK):
                            nc.tensor.matmul(
                                pv[:, 0:256],
                                xT[k][:, 128 * tt:128 * (tt + 1)],
                                wv[k],
                                start=(k == 0), stop=(k == NK - 1),
                            )
                        pv3 = pv.rearrange("p (h c) -> p h c", h=4)
                        nc.vector.tensor_copy(
                            vv4[:, tt, :, 0:64], pv3[:, 0:4, 0:64])

                def emit_proj_piece(tt, cc):
                    yp = ps_bg.tile([128, 512], f32, tag="bg", name="y")
                    for kd in range(2):
                        nc.tensor.matmul(
                            yp[:],
                            OT_sb[kd][:, 128 * tt:128 * (tt + 1)],
                            wp[kd][:, 512 * cc:512 * (cc + 1)],
                            start=(kd == 0), stop=(kd == 1),
                        )
                    ysb = ypool.tile([128, 512], bf, tag="ysb", name="ysb")
                    nc.vector.tensor_copy(ysb[:], yp[:])
                    nc.sync.dma_start(
                        y_d.ap()[128 * tt:128 * (tt + 1),
                                 512 * cc:512 * (cc + 1)],
                        ysb[:])

                bg = []  # deferred background piece list

                def emit_bg_item():
                    kind, a, b_ = bg.pop(0)
                    if kind == "qkv":
                        emit_qkv_piece(a, b_)
                    else:
                        emit_proj_piece(a, b_)

                # qkv chunk 0 inline (q/k pieces first so S can start early)
                for piece in range(8):
                    emit_qkv_piece(0, piece)

                total_iters = sum(2 * I + 2 for I in range(NSB))
                iters_done = 0

                for I in range(NSB):
                    # push background work whose deadline is superblock 2c
                    if I % 2 == 0 and I // 2 + 1 < NCH:
                        c = I // 2 + 1
                        bg.extend(("qkv", c, p) for p in range(8))

                    O_ps = [ps_o.tile([128, 260], f32, tag=f"O{hf}",
                                      name=f"O{hf}") for hf in range(2)]
                    nc.vector.memset(O_ps[0][:], 0.0)
                    nc.vector.memset(O_ps[1][:], 0.0)
                    njr = 2 * I + 2
                    Ses = [None] * njr

                    def emit_S(j, I=I):
                        Sp = ps_s.tile([128, 1024], f32, tag="S", name="S")
                        Se = sepool.tile([128, 1024], bf, tag="Se", name="Se")
                        diag1 = (j == 2 * I + 1)
                        lo = 128 if diag1 else 0
                        for h in range(HPC):
                            p, hh = divmod(h, 2)
                            nc.tensor.matmul(
                                Sp[:, 256 * h + lo:256 * (h + 1)],
                                qk_sb[2 + p][64 * hh:64 * (hh + 1),
                                             128 * j:128 * (j + 1)],
                                qk_sb[p][64 * hh:64 * (hh + 1),
                                         256 * I + lo:256 * (I + 1)],
                                start=True, stop=True,
                            )
                        sp3 = Sp.rearrange("p (h n) -> p h n", h=4)
                        se3 = Se.rearrange("p (h n) -> p h n", h=4)
                        if diag1:
                            nc.scalar.activation(
                                se3[:, :, 128:256], sp3[:, :, 128:256], Exp,
                                scale=0.125)
                            for h in range(HPC):
                                nc.vector.tensor_mul(
                                    Se[:, 256 * h + 128:256 * h + 256],
                                    Se[:, 256 * h + 128:256 * h + 256], tri)
                        else:
                            nc.scalar.activation(Se[:], Sp[:], Exp, scale=0.125)
                            if j == 2 * I:
                                for h in range(HPC):
                                    nc.vector.tensor_mul(
                                        Se[:, 256 * h:256 * h + 128],
                                        Se[:, 256 * h:256 * h + 128], tri)
                        Ses[j] = Se

                    emit_S(0)
                    for j in range(njr):
                        if j + 1 < njr:
                            emit_S(j + 1)
                        Se = Ses[j]
                        for hf in range(2):
                            if hf == 0 and j > 2 * I:
                                continue
                            stop_j = 2 * I + hf
                            for h in range(HPC):
                                nc.tensor.matmul(
                                    O_ps[hf][:, 65 * h:65 * (h + 1)],
                                    Se[:, 256 * h + 128 * hf:
                                       256 * h + 128 * (hf + 1)],
                                    vv4[:, j, h, :],
                                    start=False, stop=(j == stop_j),
                                    skip_group_check=True,
                                )
                        Ses[j] = None
                        # paced background emission
                        iters_done += 1
                        want = 0
                        if bg:
                            rem = max(1, total_iters - iters_done)
                            want = max(1, (len(bg) + rem - 1) // rem) \
                                if len(bg) > rem else 1
                        for _ in range(min(want, len(bg))):
                            emit_bg_item()

                    # normalization + transpose to O^T layout
                    rl = spool.tile([128, 8], f32, tag="rl", name="rl")
                    rl3 = rl.rearrange("p (h o) -> p h o", h=8)
                    for hf in range(2):
                        op3 = O_ps[hf].rearrange("p (h c) -> p h c", h=4)
                        nc.vector.reciprocal(
                            rl3[:, 4 * hf:4 * (hf + 1), :], op3[:, :, 64:65])
                    OTp = ps_bg.tile([128, 1024], bf, tag="bg", name="OTp")
                    for hf in range(2):
                        osb = ospool.tile([128, 256], bf, tag="osb", name="osb")
                        for h in range(HPC):
                            nc.vector.tensor_scalar_mul(
                                osb[:, 64 * h:64 * (h + 1)],
                                O_ps[hf][:, 65 * h:65 * h + 64],
                                rl[:, 4 * hf + h:4 * hf + h + 1],
                            )
                        for m in range(2):
                            nc.tensor.transpose(
                                OTp[:, 256 * hf + 128 * m:
                                    256 * hf + 128 * (m + 1)],
                                osb[:, 128 * m:128 * (m + 1)], ident)
                    ot4 = OTp[:, 0:512].rearrange("p (i k c) -> p i k c",
                                                  i=2, k=2)
                    for kd in range(2):
                        nc.vector.tensor_copy(
                            OT_sb[kd][:, 256 * I:256 * (I + 1)].rearrange(
                                "p (i c) -> p i c", i=2),
                            ot4[:, 0:2, kd, 0:128])
                    # projection for this superblock's token tiles rides later
                    bg.extend(("proj", tt, cc)
                              for tt in (2 * I, 2 * I + 1) for cc in range(2))

                while bg:
                    emit_bg_item()

    nc.compile()
    return nc


def make_core_inputs(x, w_qkv, b_qkv, w_proj, core, T=None):
    """Host-side shard/prep for one core. Returns the in_map dict."""
    if T is None:
        T = x.shape[1]
    b, g = divmod(core, 4)
    heads = [4 * g + i for i in range(HPC)]

    xT = np.ascontiguousarray(np.asarray(x[b], np.float32).T).astype(BF16)

    qcols = [w_qkv[:, h * HD:(h + 1) * HD] for h in heads]
    kcols = [w_qkv[:, C + h * HD:C + (h + 1) * HD] for h in heads]
    vcols = [w_qkv[:, 2 * C + h * HD:2 * C + (h + 1) * HD] for h in heads]
    wqk = np.concatenate(qcols + kcols, axis=1).astype(BF16)      # [C, 512]
    wv = np.concatenate(vcols, axis=1).astype(BF16)               # [C, 256]
    wp = np.concatenate([w_proj[h * HD:(h + 1) * HD, :] for h in heads],
                        axis=0).astype(BF16)                      # [256, C]

    bq = [b_qkv[h * HD:(h + 1) * HD] for h in heads]
    bk = [b_qkv[C + h * HD:C + (h + 1) * HD] for h in heads]
    bqk = np.concatenate(bq + bk).astype(np.float32).reshape(4, 128).T
    bqk = np.ascontiguousarray(bqk)                               # [128, 4]

    a = np.arange(128)
    tri = (a[:, None] <= a[None, :]).astype(BF16)                 # [128, 128]
    ident = np.eye(128, dtype=BF16)

    return {
        "xT": xT, "wqkv": np.concatenate([wqk, wv], axis=1),
        "wp": wp, "bqk": bqk,
        "cst": np.concatenate([tri, ident], axis=1),
    }


_compiled = {}


def _get_nc(T):
    if T not in _compiled:
        _compiled[T] = build_nc(T)
    return _compiled[T]


def kernel(x, w_qkv, b_qkv, w_proj, b_proj):
    from concourse.bass_utils import run_bass_kernel_spmd

    x = np.asarray(x, np.float32)
    w_qkv = np.asarray(w_qkv, np.float32)
    b_qkv = np.asarray(b_qkv, np.float32)
    w_proj = np.asarray(w_proj, np.float32)
    b_proj = np.asarray(b_proj, np.float32)
    T = x.shape[1]

    nc = _get_nc(T)
    in_maps = [make_core_inputs(x, w_qkv, b_qkv, w_proj, core, T)
               for core in range(NCORES)]
    res = run_bass_kernel_spmd(nc, in_maps, core_ids=list(range(NCORES)))

    y = np.zeros((B, T, C), np.float32)
    for core in range(NCORES):
        b = core // 4
        y[b] += res.results[core]["y"].astype(np.float32)
    y += b_proj[None, None, :] + (b_qkv[2 * C:3 * C] @ w_proj)[None, None, :]
    return y
